# revision 10
# baseline (speedup 1.0000x reference)
"""AugmentedLSTM on 8 TRN2 NeuronCores.

Sharding: gate/hidden-sharded recurrence (persistent-RNN style). Each core j
owns hidden slice j (HS=96 dims) of all 5 recurrent gates (480 rows of
w_state) plus the matching highway slice, and the full batch B=64.
Per timestep each core:
  1. waits for all 8 h-slices of step t-1 (remote_dma_broadcast mailbox)
  2. PSUM <- I64.T @ pi_t (precomputed input projection, injected via PE)
     then accumulates 8 K=96 matmuls (h-slices x w_state_slice^T)
  3. sigmoid/tanh on ACT, c/h elementwise on DVE (rows [0:n_t] only --
     lengths are compile-time constants, so the mask math is specialized out)
  4. PE-transposes its new h-slice [64,96]->[96,64], casts to bf16, and
     broadcasts it into mailbox slot j of all 8 cores (incl. itself).
The input projection x @ w_in_slice^T + bias is computed up front (fully
sharded: every core reads all of x but only its 576 rows of w_in).
"""

import os
import sys

sys.path.insert(0, "/opt/trn_rl_repo")

import numpy as np
import ml_dtypes

import concourse.bass as bass
import concourse.mybir as mybir
from concourse import bacc
from concourse.tile import TileContext

B, T, E, H = 64, 512, 768, 768
NCORES = 8
HS = H // NCORES  # 96 hidden dims per core
GS = 5 * HS  # 480 recurrent gate rows per core
PS = 6 * HS  # 576 projection rows per core (5 gates + highway)
F32 = mybir.dt.float32
BF16 = mybir.dt.bfloat16

# gate order in the packed per-core layout: ig fg og hg mi (sigmoids first)
GATE_ORDER = (0, 1, 3, 4, 2)
SIG_END = 4 * HS  # cols [0:384] sigmoid
IG = slice(0, HS)
FG = slice(HS, 2 * HS)
OG = slice(2 * HS, 3 * HS)
HG = slice(3 * HS, 4 * HS)
MI = slice(4 * HS, GS)
PI2 = slice(GS, PS)


def build(t_steps: int, n_act: list, b: int = B, t_total: int = None):
    """Build the SPMD bass program. n_act[t] = active batch rows at step t."""
    if t_total is None:
        t_total = t_steps
    nc = bacc.Bacc("TRN2", target_bir_lowering=False, debug=False)

    # ---- DRAM I/O ----
    xT = nc.dram_tensor("xT", [6, 128, b, t_total], BF16, kind="ExternalInput")
    w_in_T = nc.dram_tensor("w_in_T", [6, 128, PS], BF16, kind="ExternalInput")
    w_rec_T = nc.dram_tensor("w_rec_T", [8, HS, GS], BF16, kind="ExternalInput")
    bias_r = nc.dram_tensor("bias_r", [128, PS], F32, kind="ExternalInput")
    h0T = nc.dram_tensor("h0T", [8, HS, b], BF16, kind="ExternalInput")
    h0_j = nc.dram_tensor("h0_j", [b, HS], F32, kind="ExternalInput")
    c0_j = nc.dram_tensor("c0_j", [b, HS], F32, kind="ExternalInput")
    eye_f = nc.dram_tensor("eye_f", [b, b], F32, kind="ExternalInput")
    eye_b = nc.dram_tensor("eye_b", [b, b], BF16, kind="ExternalInput")

    y = nc.dram_tensor("y", [b, t_total, HS], F32, kind="ExternalOutput")
    hT_out = nc.dram_tensor("hT_out", [b, HS], F32, kind="ExternalOutput")
    cT_out = nc.dram_tensor("cT_out", [b, HS], F32, kind="ExternalOutput")
    proj = nc.dram_tensor("proj", [b, t_total, PS], BF16)

    # ---- persistent SBUF ----
    w_rec_sb = nc.alloc_sbuf_tensor("w_rec_sb", [HS, 8, GS], BF16)
    w_in_sb = nc.alloc_sbuf_tensor("w_in_sb", [128, 6, PS], BF16)
    bias_sb = nc.alloc_sbuf_tensor("bias_sb", [128, PS], F32)
    mail = nc.alloc_sbuf_tensor("mail", [128, 2, 8, b], BF16)
    stage = nc.alloc_sbuf_tensor("stage", [128, b], BF16)
    c_sb = nc.alloc_sbuf_tensor("c_sb", [b, HS], F32)
    h_sb = nc.alloc_sbuf_tensor("h_sb", [b, HS], F32)
    eyef_sb = nc.alloc_sbuf_tensor("eyef_sb", [b, b], F32)
    eyeb_sb = nc.alloc_sbuf_tensor("eyeb_sb", [b, b], BF16)

    rsems = [
        [nc.alloc_semaphore(f"rsem{k}_{p}", num=200 + 2 * k + p) for p in range(2)]
        for k in range(8)
    ]
    lsem = nc.alloc_semaphore("lsem", num=241)
    psem = nc.alloc_semaphore("psem", num=242)
    rdests = [(0, d) for d in range(8)]

    with TileContext(nc) as tc:
        with (
            tc.tile_pool(name="big", bufs=3) as big,
            tc.tile_pool(name="small", bufs=4) as small,
            tc.tile_pool(name="pip", bufs=6) as pip,
            tc.tile_pool(name="psum", bufs=2, space="PSUM") as psum,
            tc.tile_pool(name="psumt", bufs=2, space="PSUM") as psumt,
        ):
            # ---- preload persistent tiles ----
            nc.sync.dma_start(w_in_sb[:], w_in_T[:].rearrange("k p f -> p k f"))
            for k in range(8):
                nc.sync.dma_start(w_rec_sb[:, k, :], w_rec_T[k])
                nc.sync.dma_start(mail[0:HS, 0, k, :], h0T[k])
            nc.sync.dma_start(bias_sb[:], bias_r[:])
            nc.sync.dma_start(c_sb[:], c0_j[:])
            nc.sync.dma_start(h_sb[:], h0_j[:])
            nc.sync.dma_start(eyef_sb[:], eye_f[:])
            nc.sync.dma_start(eyeb_sb[:], eye_b[:])
            nc.vector.memset(stage[:], 0.0)

            # ---- phase 1: input projection ----
            n_tc = (t_total + 127) // 128
            for bi in range(b):
                xb = big.tile([128, 6, t_total], BF16, tag="xb")
                for k in range(6):
                    nc.sync.dma_start(xb[:, k, :], xT[k, :, bi, :])
                for ci in range(n_tc):
                    tw = min(128, t_total - ci * 128)
                    pp_full = psum.tile([128, PS], F32, tag="pp")
                    pp = pp_full[:tw]
                    for k in range(6):
                        lhsT = xb[:, k, ci * 128 : ci * 128 + tw]
                        nc.tensor.matmul(
                            pp[:, 0:512],
                            lhsT,
                            w_in_sb[:, k, 0:512],
                            start=(k == 0),
                            stop=(k == 5),
                        )
                        nc.tensor.matmul(
                            pp[:, 512:PS],
                            lhsT,
                            w_in_sb[:, k, 512:PS],
                            start=(k == 0),
                            stop=(k == 5),
                        )
                    ot_full = big.tile([128, PS], BF16, tag="ot")
                    ot = ot_full[:tw]
                    nc.vector.tensor_add(ot, pp, bias_sb[:tw])
                    nc.sync.dma_start(proj[bi, ci * 128 : ci * 128 + tw, :], ot)

            # ---- phase 2: recurrence ----
            pid = nc.gpsimd.partition_id()
            for t in range(t_steps):
                ph_in, ph_out = t % 2, (t + 1) % 2
                n_t = n_act[t]
                pi_t = pip.tile([b, PS], BF16, tag="pi")
                nc.sync.dma_start(pi_t[:], proj[:, t, :])
                pi2f = small.tile([b, HS], F32, tag="pi2f")
                nc.vector.tensor_copy(pi2f[:n_t], pi_t[:n_t, PI2])

                ps = psum.tile([b, GS], F32, tag="ps")
                with tc.tile_critical():
                    nc.tensor.matmul(
                        ps, eyeb_sb[:], pi_t[:, 0:GS], start=True, stop=False
                    )
                    for k in range(8):
                        if t > 0:
                            nc.tensor.wait_ge(rsems[k][t % 2], 2 * ((t + 1) // 2))
                        nc.tensor.matmul(
                            ps,
                            mail[0:HS, ph_in, k, :],
                            w_rec_sb[:, k, :],
                            start=False,
                            stop=(k == 7),
                        )

                gates = small.tile([b, GS], F32, tag="gates")
                nc.scalar.activation(
                    gates[:n_t, 0:SIG_END],
                    ps[:n_t, 0:SIG_END],
                    mybir.ActivationFunctionType.Sigmoid,
                )
                nc.scalar.activation(
                    gates[:n_t, SIG_END:GS],
                    ps[:n_t, SIG_END:GS],
                    mybir.ActivationFunctionType.Tanh,
                )
                t1 = small.tile([b, HS], F32, tag="t1")
                t2 = small.tile([b, HS], F32, tag="t2")
                nc.vector.tensor_mul(t1[:n_t], gates[:n_t, IG], gates[:n_t, MI])
                nc.vector.tensor_mul(t2[:n_t], gates[:n_t, FG], c_sb[:n_t])
                nc.vector.tensor_add(c_sb[:n_t], t1[:n_t], t2[:n_t])
                tct = small.tile([b, HS], F32, tag="tct")
                nc.scalar.activation(
                    tct[:n_t], c_sb[:n_t], mybir.ActivationFunctionType.Tanh
                )
                o1 = small.tile([b, HS], F32, tag="o1")
                nc.vector.tensor_mul(o1[:n_t], gates[:n_t, OG], tct[:n_t])
                nc.vector.tensor_sub(o1[:n_t], o1[:n_t], pi2f[:n_t])
                nc.vector.tensor_mul(o1[:n_t], gates[:n_t, HG], o1[:n_t])
                nc.vector.tensor_add(h_sb[:n_t], o1[:n_t], pi2f[:n_t])
                nc.sync.dma_start(y[0:n_t, t, :], h_sb[:n_t])

                if t == t_steps - 1:
                    break  # no need to hand off the last h

                pst = psumt.tile([HS, b], F32, tag="pst")
                nc.tensor.transpose(pst, h_sb[:], eyef_sb[:])
                with tc.tile_critical():
                    nc.vector.wait_ge(lsem, 16 * t)
                    nc.vector.tensor_copy(stage[0:HS, :], pst)
                with tc.tile_critical():
                    for k in nc.gpsimd.Switch(pid, 8):
                        nc.gpsimd.remote_dma_broadcast(
                            mail[:, ph_out, k, :],
                            stage[:],
                            rsems[k][ph_out],
                            lsem,
                            rdests=rdests,
                        ).then_inc(psem, 1)
                        nc.gpsimd.wait_ge(psem, t + 1)
                        nc.gpsimd.trigger_dma(count=1)

            nc.sync.dma_start(hT_out[:], h_sb[:])
            nc.sync.dma_start(cT_out[:], c_sb[:])

    nc.compile()
    return nc


def prep_inputs(x, lengths, w_in, b_in, w_state, b_state, h0, c0, t_steps):
    """Host-side slicing/transposition -> per-core in_maps."""
    f32, bf16 = np.float32, ml_dtypes.bfloat16
    b = x.shape[0]
    xT = (
        np.ascontiguousarray(x.transpose(2, 0, 1))
        .reshape(6, 128, b, x.shape[1])
        .astype(bf16)
    )
    h0T_full = h0.T.astype(bf16)  # [768, 64]
    h0T = h0T_full.reshape(8, HS, b)
    eye_f = np.eye(b, dtype=f32)
    eye_b = np.eye(b, dtype=bf16)

    in_maps = []
    for j in range(NCORES):
        rows5 = np.concatenate(
            [g * H + j * HS + np.arange(HS) for g in GATE_ORDER]
        )
        rows6 = np.concatenate([rows5, 5 * H + j * HS + np.arange(HS)])
        w_rec_T = (
            np.ascontiguousarray(w_state[rows5].T).reshape(8, HS, GS).astype(bf16)
        )
        w_in_T = (
            np.ascontiguousarray(w_in[rows6].T).reshape(6, 128, PS).astype(bf16)
        )
        bias = (
            b_in[rows6] + np.concatenate([b_state[rows5], np.zeros(HS, f32)])
        ).astype(f32)
        bias_r = np.tile(bias[None, :], (128, 1))
        in_maps.append(
            {
                "xT": xT,
                "w_in_T": w_in_T,
                "w_rec_T": w_rec_T,
                "bias_r": np.ascontiguousarray(bias_r),
                "h0T": h0T,
                "h0_j": np.ascontiguousarray(h0[:, j * HS : (j + 1) * HS]).astype(
                    f32
                ),
                "c0_j": np.ascontiguousarray(c0[:, j * HS : (j + 1) * HS]).astype(
                    f32
                ),
                "eye_f": eye_f,
                "eye_b": eye_b,
            }
        )
    return in_maps


def kernel(x, lengths, w_in, b_in, w_state, b_state, h0, c0):
    x = np.asarray(x, dtype=np.float32)
    lengths = np.asarray(lengths).astype(np.int64)
    w_in = np.asarray(w_in, dtype=np.float32)
    b_in = np.asarray(b_in, dtype=np.float32)
    w_state = np.asarray(w_state, dtype=np.float32)
    b_state = np.asarray(b_state, dtype=np.float32)
    h0 = np.asarray(h0, dtype=np.float32)
    c0 = np.asarray(c0, dtype=np.float32)

    t_steps = int(lengths.max())
    n_act = [int((lengths > t).sum()) for t in range(t_steps)]

    nc = build(t_steps, n_act, t_total=T)
    in_maps = prep_inputs(
        x, lengths, w_in, b_in, w_state, b_state, h0, c0, t_steps
    )

    trace = os.environ.get("LSTM_TRACE", "0") == "1"
    if trace:
        import types

        try:
            import antenv.axon_hooks  # noqa: F401
        except ImportError:
            from trn_agent_boot.trn_boot import _ntff_profile_via_ctypes

            mod = types.ModuleType("antenv.axon_hooks")
            hook = _ntff_profile_via_ctypes("/opt/axon/libaxon_pjrt.so")
            mod.get_axon_ntff_profile_hook = lambda: hook
            sys.modules["antenv.axon_hooks"] = mod
    import concourse.bass_utils as bu

    bu.upload_artifacts = lambda tmpdir: tmpdir
    from concourse.bass_utils import run_bass_kernel_spmd
    res = run_bass_kernel_spmd(
        nc, in_maps, core_ids=list(range(NCORES)), trace=trace
    )
    if trace and res.exec_time_ns:
        print(f"HW exec time: {res.exec_time_ns} ns")
        kernel.last_exec_ns = res.exec_time_ns
        kernel.last_trace = res.instructions_and_trace

    out = np.zeros((B, T, H), np.float32)
    hT = np.zeros((B, H), np.float32)
    cT = np.zeros((B, H), np.float32)
    for j in range(NCORES):
        r = res.results[j]
        out[:, :, j * HS : (j + 1) * HS] = r["y"]
        hT[:, j * HS : (j + 1) * HS] = r["hT_out"]
        cT[:, j * HS : (j + 1) * HS] = r["cT_out"]
    return out, hT[None], cT[None]


# revision 20
# speedup vs baseline: 1.1381x; 1.1381x over previous
"""AugmentedLSTM on 8 TRN2 NeuronCores.

Sharding: gate/hidden-sharded recurrence (persistent-RNN style). Each core j
owns hidden slice j (HS=96 dims) of all 5 recurrent gates (480 rows of
w_state) plus the matching highway slice, and the full batch B=64.
Per timestep each core:
  1. waits for all 8 h-slices of step t-1 (remote_dma_broadcast mailbox)
  2. PSUM <- I64.T @ pi_t (precomputed input projection, injected via PE)
     then accumulates 8 K=96 matmuls (h-slices x w_state_slice^T)
  3. sigmoid/tanh on ACT, c/h elementwise on DVE (rows [0:n_t] only --
     lengths are compile-time constants, so the mask math is specialized out)
  4. PE-transposes its new h-slice [64,96]->[96,64], casts to bf16, and
     broadcasts it into mailbox slot j of all 8 cores (incl. itself).
The input projection x @ w_in_slice^T + bias is computed up front (fully
sharded: every core reads all of x but only its 576 rows of w_in).
"""

import os
import sys

sys.path.insert(0, "/opt/trn_rl_repo")

import numpy as np
import ml_dtypes

import concourse.bass as bass
import concourse.mybir as mybir
from concourse import bacc
from concourse.bass import _add_dep_helper
from concourse.tile import TileContext


def _dep(after, before, reason="order"):
    _add_dep_helper(after.ins, before.ins, sync=False, reason=reason)

B, T, E, H = 64, 512, 768, 768
NCORES = 8
HS = H // NCORES  # 96 hidden dims per core
GS = 5 * HS  # 480 recurrent gate rows per core
PS = 6 * HS  # 576 projection rows per core (5 gates + highway)
F32 = mybir.dt.float32
BF16 = mybir.dt.bfloat16

# gate order in the packed per-core layout: ig fg og hg mi (sigmoids first)
GATE_ORDER = (0, 1, 3, 4, 2)
SIG_END = 4 * HS  # cols [0:384] sigmoid
IG = slice(0, HS)
FG = slice(HS, 2 * HS)
OG = slice(2 * HS, 3 * HS)
HG = slice(3 * HS, 4 * HS)
MI = slice(4 * HS, GS)
PI2 = slice(GS, PS)


def build(t_steps: int, n_act: list, b: int = B, t_total: int = None):
    """Build the SPMD bass program. n_act[t] = active batch rows at step t."""
    if t_total is None:
        t_total = t_steps
    nc = bacc.Bacc("TRN2", target_bir_lowering=False, debug=False)

    # ---- DRAM I/O ----
    xT = nc.dram_tensor("xT", [6, 128, b, t_total], BF16, kind="ExternalInput")
    w_in_T = nc.dram_tensor("w_in_T", [6, 128, PS], BF16, kind="ExternalInput")
    w_rec_T = nc.dram_tensor("w_rec_T", [8, HS, GS], BF16, kind="ExternalInput")
    bias_r = nc.dram_tensor("bias_r", [128, PS], F32, kind="ExternalInput")
    h0T = nc.dram_tensor("h0T", [8, HS, b], BF16, kind="ExternalInput")
    h0_j = nc.dram_tensor("h0_j", [b, HS], F32, kind="ExternalInput")
    c0_j = nc.dram_tensor("c0_j", [b, HS], F32, kind="ExternalInput")
    eye_f = nc.dram_tensor("eye_f", [b, b], F32, kind="ExternalInput")
    eye_b = nc.dram_tensor("eye_b", [b, b], BF16, kind="ExternalInput")

    y = nc.dram_tensor("y", [b, t_total, HS], F32, kind="ExternalOutput")
    hT_out = nc.dram_tensor("hT_out", [b, HS], F32, kind="ExternalOutput")
    cT_out = nc.dram_tensor("cT_out", [b, HS], F32, kind="ExternalOutput")
    proj = nc.dram_tensor("proj", [b, t_total, PS], BF16)

    # ---- persistent SBUF ----
    w_rec_sb = nc.alloc_sbuf_tensor("w_rec_sb", [HS, 8, GS], BF16)
    w_in_sb = nc.alloc_sbuf_tensor("w_in_sb", [128, 6, PS], BF16)
    bias_sb = nc.alloc_sbuf_tensor("bias_sb", [128, PS], F32)
    mail = nc.alloc_sbuf_tensor("mail", [128, 2, 8, b], BF16)
    stage = nc.alloc_sbuf_tensor("stage", [128, b], BF16)
    c_sb = nc.alloc_sbuf_tensor("c_sb", [b, HS], F32)
    h_sb = nc.alloc_sbuf_tensor("h_sb", [b, HS], F32)
    eyef_sb = nc.alloc_sbuf_tensor("eyef_sb", [b, b], F32)
    eyeb_sb = nc.alloc_sbuf_tensor("eyeb_sb", [b, b], BF16)

    rsems = [
        [nc.alloc_semaphore(f"rsem{k}_{p}", num=200 + 2 * k + p) for p in range(2)]
        for k in range(8)
    ]
    lsem = nc.alloc_semaphore("lsem", num=241)
    psem = nc.alloc_semaphore("psem", num=242)
    csem = nc.alloc_semaphore("csem", num=243)
    rdests = [(0, d) for d in range(8)]

    with TileContext(nc) as tc:
        with (
            tc.tile_pool(name="big", bufs=3) as big,
            tc.tile_pool(name="small", bufs=4) as small,
            tc.tile_pool(name="pip", bufs=6) as pip,
            tc.tile_pool(name="psum", bufs=2, space="PSUM") as psum,
            tc.tile_pool(name="psumt", bufs=2, space="PSUM") as psumt,
        ):
            # ---- preload persistent tiles ----
            nc.sync.dma_start(w_in_sb[:], w_in_T[:].rearrange("k p f -> p k f"))
            for k in range(8):
                nc.sync.dma_start(w_rec_sb[:, k, :], w_rec_T[k])
                nc.sync.dma_start(mail[0:HS, 0, k, :], h0T[k])
            nc.sync.dma_start(bias_sb[:], bias_r[:])
            nc.sync.dma_start(c_sb[:], c0_j[:])
            nc.sync.dma_start(h_sb[:], h0_j[:])
            nc.sync.dma_start(eyef_sb[:], eye_f[:])
            nc.sync.dma_start(eyeb_sb[:], eye_b[:])
            nc.vector.memset(stage[:], 0.0)

            # ---- phase 1: input projection ----
            n_tc = (t_total + 127) // 128
            for bi in range(b):
                xb = big.tile([128, 6, t_total], BF16, tag="xb")
                for k in range(6):
                    nc.sync.dma_start(xb[:, k, :], xT[k, :, bi, :])
                for ci in range(n_tc):
                    tw = min(128, t_total - ci * 128)
                    pp_full = psum.tile([128, PS], F32, tag="pp")
                    pp = pp_full[:tw]
                    for k in range(6):
                        lhsT = xb[:, k, ci * 128 : ci * 128 + tw]
                        nc.tensor.matmul(
                            pp[:, 0:512],
                            lhsT,
                            w_in_sb[:, k, 0:512],
                            start=(k == 0),
                            stop=(k == 5),
                        )
                        nc.tensor.matmul(
                            pp[:, 512:PS],
                            lhsT,
                            w_in_sb[:, k, 512:PS],
                            start=(k == 0),
                            stop=(k == 5),
                        )
                    ot_full = big.tile([128, PS], BF16, tag="ot")
                    ot = ot_full[:tw]
                    nc.vector.tensor_add(ot, pp, bias_sb[:tw])
                    nc.sync.dma_start(proj[bi, ci * 128 : ci * 128 + tw, :], ot)

            # ---- phase 2: recurrence ----
            pid = nc.gpsimd.partition_id()
            for t in range(t_steps):
                ph_in, ph_out = t % 2, (t + 1) % 2
                n_t = n_act[t]
                pi_t = pip.tile([b, PS], BF16, tag="pi")
                nc.sync.dma_start(pi_t[:], proj[:, t, :])
                pi2f = small.tile([b, HS], F32, tag="pi2f")
                nc.vector.tensor_copy(pi2f[:n_t], pi_t[:n_t, PI2])

                ps = psum.tile([b, GS], F32, tag="ps")
                with tc.tile_critical():
                    nc.tensor.matmul(
                        ps, eyeb_sb[:], pi_t[:, 0:GS], start=True, stop=False
                    )
                    for k in range(8):
                        if t > 0:
                            nc.tensor.wait_ge(rsems[k][t % 2], 2 * ((t + 1) // 2))
                        nc.tensor.matmul(
                            ps,
                            mail[0:HS, ph_in, k, :],
                            w_rec_sb[:, k, :],
                            start=False,
                            stop=(k == 7),
                        )

                gates = small.tile([b, GS], F32, tag="gates")
                nc.scalar.activation(
                    gates[:n_t, 0:SIG_END],
                    ps[:n_t, 0:SIG_END],
                    mybir.ActivationFunctionType.Sigmoid,
                )
                nc.scalar.activation(
                    gates[:n_t, SIG_END:GS],
                    ps[:n_t, SIG_END:GS],
                    mybir.ActivationFunctionType.Tanh,
                )
                t1 = small.tile([b, HS], F32, tag="t1")
                t2 = small.tile([b, HS], F32, tag="t2")
                nc.vector.tensor_mul(t1[:n_t], gates[:n_t, IG], gates[:n_t, MI])
                nc.vector.tensor_mul(t2[:n_t], gates[:n_t, FG], c_sb[:n_t])
                nc.vector.tensor_add(c_sb[:n_t], t1[:n_t], t2[:n_t])
                tct = small.tile([b, HS], F32, tag="tct")
                nc.scalar.activation(
                    tct[:n_t], c_sb[:n_t], mybir.ActivationFunctionType.Tanh
                )
                o1 = small.tile([b, HS], F32, tag="o1")
                nc.vector.tensor_mul(o1[:n_t], gates[:n_t, OG], tct[:n_t])
                nc.vector.tensor_sub(o1[:n_t], o1[:n_t], pi2f[:n_t])
                nc.vector.tensor_mul(o1[:n_t], gates[:n_t, HG], o1[:n_t])
                h_add = nc.vector.tensor_add(h_sb[:n_t], o1[:n_t], pi2f[:n_t])
                nc.sync.dma_start(y[0:n_t, t, :], h_sb[:n_t])

                if t == t_steps - 1:
                    break  # no need to hand off the last h

                pst = psumt.tile([HS, b], F32, tag="pst")
                nc.tensor.transpose(pst, h_sb[:], eyef_sb[:])
                with tc.tile_critical():
                    nc.vector.wait_ge(lsem, 16 * t)
                    nc.vector.tensor_copy(stage[0:HS, :], pst)
                with tc.tile_critical():
                    for k in nc.gpsimd.Switch(pid, 8):
                        nc.gpsimd.remote_dma_broadcast(
                            mail[:, ph_out, k, :],
                            stage[:],
                            rsems[k][ph_out],
                            lsem,
                            rdests=rdests,
                        ).then_inc(psem, 1)
                        nc.gpsimd.wait_ge(psem, t + 1)
                        nc.gpsimd.trigger_dma(count=1)

            nc.sync.dma_start(hT_out[:], h_sb[:])
            nc.sync.dma_start(cT_out[:], c_sb[:])

    nc.compile()
    return nc


def prep_inputs(x, lengths, w_in, b_in, w_state, b_state, h0, c0, t_steps):
    """Host-side slicing/transposition -> per-core in_maps."""
    f32, bf16 = np.float32, ml_dtypes.bfloat16
    b = x.shape[0]
    xT = (
        np.ascontiguousarray(x.transpose(2, 0, 1))
        .reshape(6, 128, b, x.shape[1])
        .astype(bf16)
    )
    h0T_full = h0.T.astype(bf16)  # [768, 64]
    h0T = h0T_full.reshape(8, HS, b)
    eye_f = np.eye(b, dtype=f32)
    eye_b = np.eye(b, dtype=bf16)

    in_maps = []
    for j in range(NCORES):
        rows5 = np.concatenate(
            [g * H + j * HS + np.arange(HS) for g in GATE_ORDER]
        )
        rows6 = np.concatenate([rows5, 5 * H + j * HS + np.arange(HS)])
        w_rec_T = (
            np.ascontiguousarray(w_state[rows5].T).reshape(8, HS, GS).astype(bf16)
        )
        w_in_T = (
            np.ascontiguousarray(w_in[rows6].T).reshape(6, 128, PS).astype(bf16)
        )
        bias = (
            b_in[rows6] + np.concatenate([b_state[rows5], np.zeros(HS, f32)])
        ).astype(f32)
        bias_r = np.tile(bias[None, :], (128, 1))
        in_maps.append(
            {
                "xT": xT,
                "w_in_T": w_in_T,
                "w_rec_T": w_rec_T,
                "bias_r": np.ascontiguousarray(bias_r),
                "h0T": h0T,
                "h0_j": np.ascontiguousarray(h0[:, j * HS : (j + 1) * HS]).astype(
                    f32
                ),
                "c0_j": np.ascontiguousarray(c0[:, j * HS : (j + 1) * HS]).astype(
                    f32
                ),
                "eye_f": eye_f,
                "eye_b": eye_b,
            }
        )
    return in_maps


def kernel(x, lengths, w_in, b_in, w_state, b_state, h0, c0):
    x = np.asarray(x, dtype=np.float32)
    lengths = np.asarray(lengths).astype(np.int64)
    w_in = np.asarray(w_in, dtype=np.float32)
    b_in = np.asarray(b_in, dtype=np.float32)
    w_state = np.asarray(w_state, dtype=np.float32)
    b_state = np.asarray(b_state, dtype=np.float32)
    h0 = np.asarray(h0, dtype=np.float32)
    c0 = np.asarray(c0, dtype=np.float32)

    t_steps = int(lengths.max())
    n_act = [int((lengths > t).sum()) for t in range(t_steps)]

    nc = build(t_steps, n_act, t_total=T)
    in_maps = prep_inputs(
        x, lengths, w_in, b_in, w_state, b_state, h0, c0, t_steps
    )

    trace = os.environ.get("LSTM_TRACE", "0") == "1"
    if trace:
        import types

        try:
            import antenv.axon_hooks  # noqa: F401
        except ImportError:
            from trn_agent_boot.trn_boot import _ntff_profile_via_ctypes

            mod = types.ModuleType("antenv.axon_hooks")
            hook = _ntff_profile_via_ctypes("/opt/axon/libaxon_pjrt.so")
            mod.get_axon_ntff_profile_hook = lambda: hook
            sys.modules["antenv.axon_hooks"] = mod
    import concourse.bass_utils as bu

    bu.upload_artifacts = lambda tmpdir: tmpdir
    from concourse.bass_utils import run_bass_kernel_spmd
    res = run_bass_kernel_spmd(
        nc, in_maps, core_ids=list(range(NCORES)), trace=trace
    )
    if trace and res.exec_time_ns:
        print(f"HW exec time: {res.exec_time_ns} ns")
        kernel.last_exec_ns = res.exec_time_ns
        kernel.last_trace = res.instructions_and_trace

    out = np.zeros((B, T, H), np.float32)
    hT = np.zeros((B, H), np.float32)
    cT = np.zeros((B, H), np.float32)
    for j in range(NCORES):
        r = res.results[j]
        out[:, :, j * HS : (j + 1) * HS] = r["y"]
        hT[:, j * HS : (j + 1) * HS] = r["hT_out"]
        cT[:, j * HS : (j + 1) * HS] = r["cT_out"]
    return out, hT[None], cT[None]


# revision 23
# speedup vs baseline: 1.3401x; 1.1775x over previous
"""AugmentedLSTM on 8 TRN2 NeuronCores.

Sharding: gate/hidden-sharded recurrence (persistent-RNN style). Each core j
owns hidden slice j (HS=96 dims) of all 5 recurrent gates (480 rows of
w_state) plus the matching highway slice, and the full batch B=64.
Per timestep each core:
  1. waits for all 8 h-slices of step t-1 (remote_dma_broadcast mailbox)
  2. PSUM <- I64.T @ pi_t (precomputed input projection, injected via PE)
     then accumulates 8 K=96 matmuls (h-slices x w_state_slice^T)
  3. sigmoid/tanh on ACT, c/h elementwise on DVE (rows [0:n_t] only --
     lengths are compile-time constants, so the mask math is specialized out)
  4. PE-transposes its new h-slice [64,96]->[96,64], casts to bf16, and
     broadcasts it into mailbox slot j of all 8 cores (incl. itself).
The input projection x @ w_in_slice^T + bias is computed up front (fully
sharded: every core reads all of x but only its 576 rows of w_in).
"""

import os
import sys

sys.path.insert(0, "/opt/trn_rl_repo")

import numpy as np
import ml_dtypes

import concourse.bass as bass
import concourse.mybir as mybir
from concourse import bacc
from concourse.bass import _add_dep_helper
from concourse.tile import TileContext


def _dep(after, before, reason="order"):
    _add_dep_helper(after.ins, before.ins, sync=False, reason=reason)

B, T, E, H = 64, 512, 768, 768
NCORES = 8
HS = H // NCORES  # 96 hidden dims per core
GS = 5 * HS  # 480 recurrent gate rows per core
PS = 6 * HS  # 576 projection rows per core (5 gates + highway)
F32 = mybir.dt.float32
BF16 = mybir.dt.bfloat16

# gate order in the packed per-core layout: ig fg og hg mi (sigmoids first)
GATE_ORDER = (0, 1, 3, 4, 2)
SIG_END = 4 * HS  # cols [0:384] sigmoid
IG = slice(0, HS)
FG = slice(HS, 2 * HS)
OG = slice(2 * HS, 3 * HS)
HG = slice(3 * HS, 4 * HS)
MI = slice(4 * HS, GS)
PI2 = slice(GS, PS)


def build(t_steps: int, n_act: list, b: int = B, t_total: int = None):
    """Build the SPMD bass program. n_act[t] = active batch rows at step t."""
    if t_total is None:
        t_total = t_steps
    nc = bacc.Bacc("TRN2", target_bir_lowering=False, debug=False)

    # ---- DRAM I/O ----
    xT = nc.dram_tensor("xT", [6, 128, b, t_total], BF16, kind="ExternalInput")
    w_in_T = nc.dram_tensor("w_in_T", [6, 128, PS], BF16, kind="ExternalInput")
    w_rec_T = nc.dram_tensor("w_rec_T", [8, HS, GS], BF16, kind="ExternalInput")
    bias_r = nc.dram_tensor("bias_r", [128, PS], F32, kind="ExternalInput")
    h0T = nc.dram_tensor("h0T", [8, HS, b], BF16, kind="ExternalInput")
    h0_j = nc.dram_tensor("h0_j", [b, HS], F32, kind="ExternalInput")
    c0_j = nc.dram_tensor("c0_j", [b, HS], F32, kind="ExternalInput")
    eye_f = nc.dram_tensor("eye_f", [b, b], F32, kind="ExternalInput")
    eye_b = nc.dram_tensor("eye_b", [b, b], BF16, kind="ExternalInput")

    y = nc.dram_tensor("y", [b, t_total, HS], F32, kind="ExternalOutput")
    hT_out = nc.dram_tensor("hT_out", [b, HS], F32, kind="ExternalOutput")
    cT_out = nc.dram_tensor("cT_out", [b, HS], F32, kind="ExternalOutput")
    proj = nc.dram_tensor("proj", [b, t_total, PS], BF16)

    # ---- persistent SBUF ----
    w_rec_sb = nc.alloc_sbuf_tensor("w_rec_sb", [HS, 8, GS], BF16)
    w_in_sb = nc.alloc_sbuf_tensor("w_in_sb", [128, 6, PS], BF16)
    bias_sb = nc.alloc_sbuf_tensor("bias_sb", [128, PS], F32)
    mail = nc.alloc_sbuf_tensor("mail", [128, 2, 8, b], BF16)
    stage = nc.alloc_sbuf_tensor("stage", [128, b], BF16)
    c_sb = nc.alloc_sbuf_tensor("c_sb", [b, HS], F32)
    h_sb = nc.alloc_sbuf_tensor("h_sb", [b, HS], F32)
    eyef_sb = nc.alloc_sbuf_tensor("eyef_sb", [b, b], F32)
    eyeb_sb = nc.alloc_sbuf_tensor("eyeb_sb", [b, b], BF16)

    rsems = [
        [nc.alloc_semaphore(f"rsem{k}_{p}", num=200 + 2 * k + p) for p in range(2)]
        for k in range(8)
    ]
    lsem = nc.alloc_semaphore("lsem", num=241)
    psem = nc.alloc_semaphore("psem", num=242)
    csem = nc.alloc_semaphore("csem", num=243)
    rdests = [(0, d) for d in range(8)]

    with TileContext(nc) as tc:
        with (
            tc.tile_pool(name="big", bufs=3) as big,
            tc.tile_pool(name="small", bufs=4) as small,
            tc.tile_pool(name="pip", bufs=6) as pip,
            tc.tile_pool(name="psum", bufs=2, space="PSUM") as psum,
            tc.tile_pool(name="psumt", bufs=2, space="PSUM") as psumt,
        ):
            # ---- preload persistent tiles ----
            nc.sync.dma_start(w_in_sb[:], w_in_T[:].rearrange("k p f -> p k f"))
            for k in range(8):
                nc.sync.dma_start(w_rec_sb[:, k, :], w_rec_T[k])
                nc.sync.dma_start(mail[0:HS, 0, k, :], h0T[k])
            nc.sync.dma_start(bias_sb[:], bias_r[:])
            nc.sync.dma_start(c_sb[:], c0_j[:])
            nc.sync.dma_start(h_sb[:], h0_j[:])
            nc.sync.dma_start(eyef_sb[:], eye_f[:])
            nc.sync.dma_start(eyeb_sb[:], eye_b[:])
            nc.vector.memset(stage[:], 0.0)

            # ---- phase 1: input projection ----
            n_tc = (t_total + 127) // 128
            for bi in range(b):
                xb = big.tile([128, 6, t_total], BF16, tag="xb")
                for k in range(6):
                    nc.sync.dma_start(xb[:, k, :], xT[k, :, bi, :])
                for ci in range(n_tc):
                    tw = min(128, t_total - ci * 128)
                    pp_full = psum.tile([128, PS], F32, tag="pp")
                    pp = pp_full[:tw]
                    for k in range(6):
                        lhsT = xb[:, k, ci * 128 : ci * 128 + tw]
                        nc.tensor.matmul(
                            pp[:, 0:512],
                            lhsT,
                            w_in_sb[:, k, 0:512],
                            start=(k == 0),
                            stop=(k == 5),
                        )
                        nc.tensor.matmul(
                            pp[:, 512:PS],
                            lhsT,
                            w_in_sb[:, k, 512:PS],
                            start=(k == 0),
                            stop=(k == 5),
                        )
                    ot_full = big.tile([128, PS], BF16, tag="ot")
                    ot = ot_full[:tw]
                    nc.vector.tensor_add(ot, pp, bias_sb[:tw])
                    nc.sync.dma_start(proj[bi, ci * 128 : ci * 128 + tw, :], ot)

            # ---- phase 2: recurrence ----
            pid = nc.gpsimd.partition_id()
            for t in range(t_steps):
                ph_in, ph_out = t % 2, (t + 1) % 2
                n_t = n_act[t]
                pi_t = pip.tile([b, PS], BF16, tag="pi")
                nc.sync.dma_start(pi_t[:], proj[:, t, :])
                pi2f = small.tile([b, HS], F32, tag="pi2f")
                nc.vector.tensor_copy(pi2f[:n_t], pi_t[:n_t, PI2])

                ps = psum.tile([b, GS], F32, tag="ps")
                with tc.tile_critical():
                    nc.tensor.matmul(
                        ps, eyeb_sb[:], pi_t[:, 0:GS], start=True, stop=False
                    )
                    for k in range(8):
                        if t > 0:
                            nc.tensor.wait_ge(rsems[k][t % 2], 2 * ((t + 1) // 2))
                        nc.tensor.matmul(
                            ps,
                            mail[0:HS, ph_in, k, :],
                            w_rec_sb[:, k, :],
                            start=False,
                            stop=(k == 7),
                        )

                gates = small.tile([b, GS], F32, tag="gates")
                nc.scalar.activation(
                    gates[:n_t, 0:SIG_END],
                    ps[:n_t, 0:SIG_END],
                    mybir.ActivationFunctionType.Sigmoid,
                )
                nc.scalar.activation(
                    gates[:n_t, SIG_END:GS],
                    ps[:n_t, SIG_END:GS],
                    mybir.ActivationFunctionType.Tanh,
                )
                t1 = small.tile([b, HS], F32, tag="t1")
                t2 = small.tile([b, HS], F32, tag="t2")
                nc.vector.tensor_mul(t1[:n_t], gates[:n_t, IG], gates[:n_t, MI])
                nc.vector.tensor_mul(t2[:n_t], gates[:n_t, FG], c_sb[:n_t])
                nc.vector.tensor_add(c_sb[:n_t], t1[:n_t], t2[:n_t])
                tct = small.tile([b, HS], F32, tag="tct")
                nc.scalar.activation(
                    tct[:n_t], c_sb[:n_t], mybir.ActivationFunctionType.Tanh
                )
                o1 = small.tile([b, HS], F32, tag="o1")
                nc.vector.tensor_mul(o1[:n_t], gates[:n_t, OG], tct[:n_t])
                nc.vector.tensor_sub(o1[:n_t], o1[:n_t], pi2f[:n_t])
                nc.vector.tensor_mul(o1[:n_t], gates[:n_t, HG], o1[:n_t])
                h_add = nc.vector.tensor_add(h_sb[:n_t], o1[:n_t], pi2f[:n_t])
                nc.sync.dma_start(y[0:n_t, t, :], h_sb[:n_t])

                if t == t_steps - 1:
                    break  # no need to hand off the last h

                pst = psumt.tile([HS, b], F32, tag="pst")
                nc.tensor.transpose(pst, h_sb[:], eyef_sb[:])
                with tc.tile_critical():
                    nc.vector.wait_ge(lsem, 16 * t)
                    cast = nc.vector.tensor_copy(stage[0:HS, :], pst)
                    cast.then_inc(csem, 1)
                    # order the gpsimd descriptor read after the DVE write --
                    # inside one critical Tile does not auto-sync across engines
                    nc.gpsimd.wait_ge(csem, t + 1)
                    for k in nc.gpsimd.Switch(pid, 8):
                        nc.gpsimd.remote_dma_broadcast(
                            mail[:, ph_out, k, :],
                            stage[:],
                            rsems[k][ph_out],
                            lsem,
                            rdests=rdests,
                        ).then_inc(psem, 1)
                        nc.gpsimd.wait_ge(psem, t + 1)
                        nc.gpsimd.trigger_dma(count=1)

            nc.sync.dma_start(hT_out[:], h_sb[:])
            nc.sync.dma_start(cT_out[:], c_sb[:])

    nc.compile()
    return nc


def prep_inputs(x, lengths, w_in, b_in, w_state, b_state, h0, c0, t_steps):
    """Host-side slicing/transposition -> per-core in_maps."""
    f32, bf16 = np.float32, ml_dtypes.bfloat16
    b = x.shape[0]
    xT = (
        np.ascontiguousarray(x.transpose(2, 0, 1))
        .reshape(6, 128, b, x.shape[1])
        .astype(bf16)
    )
    h0T_full = h0.T.astype(bf16)  # [768, 64]
    h0T = h0T_full.reshape(8, HS, b)
    eye_f = np.eye(b, dtype=f32)
    eye_b = np.eye(b, dtype=bf16)

    in_maps = []
    for j in range(NCORES):
        rows5 = np.concatenate(
            [g * H + j * HS + np.arange(HS) for g in GATE_ORDER]
        )
        rows6 = np.concatenate([rows5, 5 * H + j * HS + np.arange(HS)])
        w_rec_T = (
            np.ascontiguousarray(w_state[rows5].T).reshape(8, HS, GS).astype(bf16)
        )
        w_in_T = (
            np.ascontiguousarray(w_in[rows6].T).reshape(6, 128, PS).astype(bf16)
        )
        bias = (
            b_in[rows6] + np.concatenate([b_state[rows5], np.zeros(HS, f32)])
        ).astype(f32)
        bias_r = np.tile(bias[None, :], (128, 1))
        in_maps.append(
            {
                "xT": xT,
                "w_in_T": w_in_T,
                "w_rec_T": w_rec_T,
                "bias_r": np.ascontiguousarray(bias_r),
                "h0T": h0T,
                "h0_j": np.ascontiguousarray(h0[:, j * HS : (j + 1) * HS]).astype(
                    f32
                ),
                "c0_j": np.ascontiguousarray(c0[:, j * HS : (j + 1) * HS]).astype(
                    f32
                ),
                "eye_f": eye_f,
                "eye_b": eye_b,
            }
        )
    return in_maps


def kernel(x, lengths, w_in, b_in, w_state, b_state, h0, c0):
    x = np.asarray(x, dtype=np.float32)
    lengths = np.asarray(lengths).astype(np.int64)
    w_in = np.asarray(w_in, dtype=np.float32)
    b_in = np.asarray(b_in, dtype=np.float32)
    w_state = np.asarray(w_state, dtype=np.float32)
    b_state = np.asarray(b_state, dtype=np.float32)
    h0 = np.asarray(h0, dtype=np.float32)
    c0 = np.asarray(c0, dtype=np.float32)

    t_steps = int(lengths.max())
    n_act = [int((lengths > t).sum()) for t in range(t_steps)]

    nc = build(t_steps, n_act, t_total=T)
    in_maps = prep_inputs(
        x, lengths, w_in, b_in, w_state, b_state, h0, c0, t_steps
    )

    trace = os.environ.get("LSTM_TRACE", "0") == "1"
    if trace:
        import types

        try:
            import antenv.axon_hooks  # noqa: F401
        except ImportError:
            from trn_agent_boot.trn_boot import _ntff_profile_via_ctypes

            mod = types.ModuleType("antenv.axon_hooks")
            hook = _ntff_profile_via_ctypes("/opt/axon/libaxon_pjrt.so")
            mod.get_axon_ntff_profile_hook = lambda: hook
            sys.modules["antenv.axon_hooks"] = mod
    import concourse.bass_utils as bu

    bu.upload_artifacts = lambda tmpdir: tmpdir
    from concourse.bass_utils import run_bass_kernel_spmd
    res = run_bass_kernel_spmd(
        nc, in_maps, core_ids=list(range(NCORES)), trace=trace
    )
    if trace and res.exec_time_ns:
        print(f"HW exec time: {res.exec_time_ns} ns")
        kernel.last_exec_ns = res.exec_time_ns
        kernel.last_trace = res.instructions_and_trace

    out = np.zeros((B, T, H), np.float32)
    hT = np.zeros((B, H), np.float32)
    cT = np.zeros((B, H), np.float32)
    for j in range(NCORES):
        r = res.results[j]
        out[:, :, j * HS : (j + 1) * HS] = r["y"]
        hT[:, j * HS : (j + 1) * HS] = r["hT_out"]
        cT[:, j * HS : (j + 1) * HS] = r["cT_out"]
    return out, hT[None], cT[None]
